# revision 17
# baseline (speedup 1.0000x reference)
"""Multi-head attention (no softmax) on 8 trn2 NeuronCores.

Reference: out = ((x @ Wqkv.T -> q,k,v per head) ; (q @ k.T * s) @ v ; concat ; @ Wproj.T)

Because there is no softmax the attention is linear:
    (q @ k.T) @ v == q @ (k.T @ v),  k.T @ v is only 64x64 per head,
so the T x T score matrices never need to exist. Per head:
    M_h = (s * k_h).T @ v_h        (64 x 64, reduced over ALL tokens of the batch)
    out += (q_h @ M_h) @ Wproj_h.T

Sharding: token-parallel. Core c owns batch b=c//2, token half c%2 (512 tokens).
M_h needs a reduction over the full batch -> two tiny 64KB AllGathers between
the two cores of each batch (peer-add done locally on DVE), overlapped with
the second kv half and the q matmuls.

The whole datapath runs in bf16 (fp32 PSUM accumulation): rel err ~5e-3 vs
the 2e-2 gate, and it halves HBM traffic (~10MB/core) so the kernel is
tensor-bound. The head-dim scale 1/8 is folded into W_k on the host (exact).

All inputs are host-packed into [128, N] DRAM tensors whose rows are fully
contiguous (4KB per partition line), so every DMA runs near full HBM rate.
One SBUF tile per DMA (whole-tile write deps would otherwise serialize
back-to-back transfers into the same tile):
  xP   (128, 4096):  xP[p, 512e+t]         = x[tok t, feat 128e+p]
  wkvP (128, 16384): wkvP[p, (P*8+e)512+c] = chunk_P.T[128e+p, c], chunks in
                     order (k half0, v half0, k half1, v half1), features
                     grouped h*64+j, k pre-scaled by 1/8
  wqP  (128, 8192):  wqP[p, 1024e+c]       = Wq_g.T[128e+p, c]
  wpP  (128, 8192):  wpP[p, 1024f+o]       = W_proj.T[128f+p, o]

Queues: Sync = kv weight stream then collective readbacks (idle by then);
GpSimd = x, wq, wp, gather bounces + triggers, output stores; Scalar(ACT) =
half the PSUM evictions + M staging copies; DVE = the other evictions + M
peer-adds. A chain of dummy matmuls at program start ramps the PE p-state
to 2.4GHz before real work arrives.

PSUM: one 8-bank rotating pool; allocation order keeps the 8 output
accumulators simultaneously live at the end.
"""

import numpy as np

B, T, E = 4, 1024, 1024
NH, HD = 16, 64
N_CORES = 8
TPC = T // 2  # tokens per core = 512

_built = None


def _build():
    """Build + compile the 8-core SPMD Bass program once."""
    global _built
    if _built is not None:
        return _built

    import concourse.mybir as mybir
    import concourse.tile as tile
    from concourse import bacc

    f32 = mybir.dt.float32
    bf16 = mybir.dt.bfloat16
    GROUPS = [[0, 1], [2, 3], [4, 5], [6, 7]]
    ALL8 = [[0, 1, 2, 3, 4, 5, 6, 7]]

    nc = bacc.Bacc("TRN2", target_bir_lowering=False, debug=False, num_devices=N_CORES)
    xP = nc.dram_tensor("xP", [128, 8 * TPC], bf16, kind="ExternalInput").ap()
    wkvP = nc.dram_tensor("wkvP", [128, 4 * 4096], bf16, kind="ExternalInput").ap()
    wqP = nc.dram_tensor("wqP", [128, 8 * 1024], bf16, kind="ExternalInput").ap()
    wpP = nc.dram_tensor("wpP", [128, 8 * 1024], bf16, kind="ExternalInput").ap()
    out = nc.dram_tensor("out", [TPC, E], f32, kind="ExternalOutput").ap()

    def evict(i, dst, src):
        # spread PSUM->SBUF eviction copies across DVE and ACT
        if i % 2 == 0:
            nc.vector.tensor_copy(dst, src)
        else:
            nc.scalar.copy(dst, src)

    with tile.TileContext(nc) as tc:
        with (
            tc.tile_pool(name="xp", bufs=1) as xp,
            tc.tile_pool(name="wkvp", bufs=1) as wkvp,
            tc.tile_pool(name="kvp", bufs=1) as kvp,
            tc.tile_pool(name="wqp", bufs=1) as wqp,
            tc.tile_pool(name="wpp", bufs=1) as wpp,
            tc.tile_pool(name="qp", bufs=1) as qp,
            tc.tile_pool(name="mres", bufs=1) as mres,
            tc.tile_pool(name="op", bufs=3) as op,
            tc.tile_pool(name="dram", bufs=1, space="DRAM") as dram,
            tc.tile_pool(name="ps", bufs=8, space="PSUM") as ps_pool,
        ):
            # ---- PE warmup: ramp the p-state while the preamble/DMAs run ----
            scratch = xp.tile([128, 640], bf16, tag="scratch")
            nc.vector.memset(scratch[:].bitcast(f32), 0.0)
            wps = ps_pool.tile([128, 512], f32, tag="big", name="warmup")
            for _ in range(9):
                nc.tensor.matmul(wps[:], scratch[:, 0:128], scratch[:, 128:640],
                                 start=True, stop=True)

            # ---- input DMAs: one SBUF tile per transfer ----
            # Sync: kv weight stream (fc order k0, v0, k1, v1; 2 tiles per fc).
            # GpSimd: x then wq (wp is issued later, after the first gather's
            # trigger, so the trigger isn't queued behind it). Scalar/DVE carry
            # NO transfers: a DMA trigger occupies its queue for the whole
            # transfer and would block the PSUM evictions behind it.
            # Split the kv stream across both rings: sync carries k0/v0 (the
            # first gather's inputs) then wq/wp; gpsimd carries x then k1/v1.
            wkvsb = []  # 8 tiles of [128, 2048]: idx 2*pos + (e>=4)
            for h in range(8):
                t = wkvp.tile([128, 2048], bf16, tag=f"wkv{h}")
                eng = nc.sync if h < 4 else nc.gpsimd
                eng.dma_start(t[:], wkvP[:, 2048 * h:2048 * (h + 1)])
                wkvsb.append(t)
            xsb = []  # 2 tiles of [128, 2048]: e 0-3, e 4-7
            for h in range(2):
                t = xp.tile([128, 2048], bf16, tag=f"x{h}")
                nc.gpsimd.dma_start(t[:], xP[:, 2048 * h:2048 * (h + 1)])
                xsb.append(t)
            wq = []  # 2 tiles of [128, 4096]: e 0-3, e 4-7
            for h in range(2):
                t = wqp.tile([128, 4096], bf16, tag=f"wq{h}")
                nc.sync.dma_start(t[:], wqP[:, 4096 * h:4096 * (h + 1)])
                wq.append(t)
            wp = []  # 2 tiles of [128, 4096]: f 0-3, f 4-7
            for h in range(2):
                t = wpp.tile([128, 4096], bf16, tag=f"wp{h}")
                nc.sync.dma_start(t[:], wpP[:, 4096 * h:4096 * (h + 1)])
                wp.append(t)

            def xs(e, lo, hi):  # x slice [128, hi-lo] for E-chunk e, token cols lo:hi
                return xsb[e // 4][:, 512 * (e % 4) + lo:512 * (e % 4) + hi]

            def wkvs(pos, e):  # wkv slice [128, 512] for fc position pos, E-chunk e
                return wkvsb[2 * pos + e // 4][:, 512 * (e % 4):512 * (e % 4 + 1)]

            kvsb = [kvp.tile([128, 2 * E], bf16, tag=f"kv{tt}", name=f"kv{tt}")
                    for tt in range(4)]
            Mbd = mres.tile([128, 1024], bf16, tag="Mbd")
            nc.vector.memset(Mbd[:].bitcast(f32), 0.0)

            bout = [None, None]

            # wkvP chunk position of each fc: fc0->0, fc2->1, fc1->2, fc3->3
            FCPOS = {0: 0, 2: 1, 1: 2, 3: 3}

            def kv_quarter(fc):
                pos = FCPOS[fc]
                for tt in range(4):
                    ps = ps_pool.tile([128, 512], f32, tag="big")
                    for e in range(8):
                        nc.tensor.matmul(
                            ps[:],
                            xs(e, 128 * tt, 128 * (tt + 1)),
                            wkvs(pos, e),
                            start=(e == 0), stop=(e == 7),
                        )
                    evict(tt, kvsb[tt][:, 512 * fc:512 * (fc + 1)], ps[:])

            def m_half(g):
                # M blocks 4g..4g+3 from k cols [512g:512g+512], v cols
                # [E+512g : E+512g+512]; keep only diagonal 64x64 sub-blocks.
                mp = ps_pool.tile([128, 512], f32, tag="big", name=f"mp{g}")
                for j in range(4):
                    blk = 4 * g + j
                    for tt in range(4):
                        nc.tensor.matmul(
                            mp[:, 128 * j:128 * (j + 1)],
                            kvsb[tt][:, 128 * blk:128 * (blk + 1)],
                            kvsb[tt][:, E + 128 * blk:E + 128 * (blk + 1)],
                            start=(tt == 0), stop=(tt == 3),
                        )
                Msb = mres.tile([128, 256], bf16, tag=f"Msb{g}", name=f"Msb{g}")
                for j in range(4):
                    nc.scalar.copy(Msb[0:64, 64 * j:64 * j + 64],
                                   mp[0:64, 128 * j:128 * j + 64])
                    nc.scalar.copy(Msb[64:128, 64 * j:64 * j + 64],
                                   mp[64:128, 128 * j + 64:128 * (j + 1)])
                # bounce to DRAM; bounce+trigger ride GpSimd, readbacks Sync
                bin_ = dram.tile([128, 256], bf16, name=f"bin{g}")
                bo = dram.tile([256, 256], bf16, name=f"bout{g}")
                nc.scalar.dma_start(bin_[:], Msb[:])
                nc.gpsimd.collective_compute(
                    "AllGather", mybir.AluOpType.bypass, replica_groups=GROUPS,
                    ins=[bin_.opt()], outs=[bo.opt()],
                )
                MrA = mres.tile([128, 256], bf16, tag=f"MrA{g}", name=f"MrA{g}")
                MrB = mres.tile([128, 256], bf16, tag=f"MrB{g}", name=f"MrB{g}")
                nc.sync.dma_start(MrA[:], bo[0:128, :])
                nc.sync.dma_start(MrB[:], bo[128:256, :])
                bout[g] = (MrA, MrB)

            def m_post(g):
                # add both ranks' partials straight into Mbd diagonal spots
                MrA, MrB = bout[g]
                for j in range(4):
                    blk = 4 * g + j
                    nc.vector.tensor_add(
                        Mbd[0:64, 128 * blk:128 * blk + 64],
                        MrA[0:64, 64 * j:64 * j + 64],
                        MrB[0:64, 64 * j:64 * j + 64])
                    nc.vector.tensor_add(
                        Mbd[64:128, 128 * blk + 64:128 * (blk + 1)],
                        MrA[64:128, 64 * j:64 * j + 64],
                        MrB[64:128, 64 * j:64 * j + 64])

            # ---- kv + M + gathers, pipelined in halves ----
            kv_quarter(0)      # k cols 0:512
            kv_quarter(2)      # v cols 0:512
            m_half(0)          # M blocks 0-3 + AllGather #1 (in flight)
            kv_quarter(1)      # k cols 512:1024
            kv_quarter(3)      # v cols 512:1024
            m_half(1)          # M blocks 4-7 + AllGather #2 (in flight)

            # ---- q (feature-major qT, (1024f, 512t)), overlaps the gathers ----
            qsb = [qp.tile([128, TPC], bf16, tag=f"q{f}", name=f"q{f}")
                   for f in range(8)]
            for fq in range(8):
                ps = ps_pool.tile([128, 512], f32, tag="big")
                for e in range(8):
                    nc.tensor.matmul(
                        ps[:],
                        wq[e // 4][:, 1024 * (e % 4) + 128 * fq:
                                   1024 * (e % 4) + 128 * (fq + 1)],
                        xs(e, 0, 512),
                        start=(e == 0), stop=(e == 7),
                    )
                evict(fq, qsb[fq][:], ps[:])

            # ---- att: attT_blk = Mbd_blk.T @ qT_blk (in-place into q tiles) ----
            m_post(0)
            for blk in range(4):
                ps = ps_pool.tile([128, 512], f32, tag="big")
                nc.tensor.matmul(ps[:], Mbd[:, 128 * blk:128 * (blk + 1)],
                                 qsb[blk][:], start=True, stop=True)
                evict(blk, qsb[blk][:], ps[:])
            attsb = qsb

            # ---- out = attT.T @ wpP  ((512t, 1024o)) ----
            # Two groups of 4 accumulators. Group A accumulates its att 0-3
            # contributions while gather #2 is still in flight; att 4-7 (the
            # only work gated on the gather) runs in the shadow of that, then
            # group A finishes, then group B does all 8 in order.
            def out_mm(ps, f, tt, oc, start, stop):
                nc.tensor.matmul(
                    ps[:],
                    attsb[f][:, 128 * tt:128 * (tt + 1)],
                    wp[f // 4][:, 1024 * (f % 4) + 512 * oc:
                               1024 * (f % 4) + 512 * (oc + 1)],
                    start=start, stop=stop,
                )

            groupA = [(0, 0), (0, 1), (1, 0), (1, 1)]
            groupB = [(2, 0), (2, 1), (3, 0), (3, 1)]
            psA = {}
            for tt, oc in groupA:
                ps = ps_pool.tile([128, 512], f32, tag="big", name=f"oA{tt}{oc}")
                psA[(tt, oc)] = ps
                for f in range(4):
                    out_mm(ps, f, tt, oc, start=(f == 0), stop=False)

            m_post(1)
            for blk in range(4, 8):
                ps = ps_pool.tile([128, 512], f32, tag="big")
                nc.tensor.matmul(ps[:], Mbd[:, 128 * blk:128 * (blk + 1)],
                                 qsb[blk][:], start=True, stop=True)
                evict(blk, qsb[blk][:], ps[:])

            i = 0
            for tt, oc in groupA:
                ps = psA[(tt, oc)]
                for f in range(4, 8):
                    out_mm(ps, f, tt, oc, start=False, stop=(f == 7))
                ot = op.tile([128, 512], f32, tag="osb")
                evict(i, ot[:], ps[:])
                eng = nc.gpsimd if i % 2 == 0 else nc.sync
                i += 1
                eng.dma_start(
                    out[128 * tt:128 * (tt + 1), 512 * oc:512 * (oc + 1)], ot[:])
            for tt, oc in groupB:
                ps = ps_pool.tile([128, 512], f32, tag="big", name=f"oB{tt}{oc}")
                for f in range(8):
                    out_mm(ps, f, tt, oc, start=(f == 0), stop=(f == 7))
                ot = op.tile([128, 512], f32, tag="osb")
                evict(i, ot[:], ps[:])
                eng = nc.gpsimd if i % 2 == 0 else nc.sync
                i += 1
                eng.dma_start(
                    out[128 * tt:128 * (tt + 1), 512 * oc:512 * (oc + 1)], ot[:])

    nc.compile()
    _built = nc
    return nc


LAST_RESULTS = None  # BassKernelResults of the most recent kernel() call


def kernel(x: np.ndarray, W_qkv: np.ndarray, W_proj: np.ndarray) -> np.ndarray:
    global LAST_RESULTS
    import ml_dtypes
    from concourse import bass_utils

    nc = _build()
    bf16 = ml_dtypes.bfloat16

    x = np.ascontiguousarray(x, dtype=np.float32)
    W_qkv = np.ascontiguousarray(W_qkv, dtype=np.float32)
    W_proj = np.ascontiguousarray(W_proj, dtype=np.float32)

    def pack(a, width):
        # (E, width) feature-major -> [128, 8*width] with row 128e+p -> col e*width
        return np.ascontiguousarray(
            a.reshape(8, 128, width).transpose(1, 0, 2).reshape(128, 8 * width)
            .astype(bf16))

    # head-grouping permutation: grouped feature h*64+j <- original row j*16+h
    perm = np.arange(E).reshape(HD, NH).T.ravel()
    Wq_g = W_qkv[perm]
    Wk_g = W_qkv[E + perm] * np.float32(HD ** -0.5)  # exact: 1/8
    Wv_g = W_qkv[2 * E + perm]
    K = Wk_g.T  # (E, kfeat)
    V = Wv_g.T
    wkvP_np = np.ascontiguousarray(np.concatenate(
        [pack(K[:, 0:512], 512), pack(V[:, 0:512], 512),
         pack(K[:, 512:1024], 512), pack(V[:, 512:1024], 512)], axis=1))
    wqP_np = pack(Wq_g.T, 1024)
    wpP_np = pack(W_proj.T, 1024)

    in_maps = []
    for c in range(N_CORES):
        b, half = c // 2, c % 2
        x_c = x[b, half * TPC:(half + 1) * TPC, :]  # (512 tok, E)
        xP_c = np.ascontiguousarray(
            x_c.reshape(TPC, 8, 128).transpose(2, 1, 0).reshape(128, 8 * TPC)
            .astype(bf16))
        in_maps.append({"xP": xP_c, "wkvP": wkvP_np, "wqP": wqP_np, "wpP": wpP_np})

    import os as _os
    _tc = _os.environ.get("KERNEL_TRACE_CORES")
    _kw = {"trace_cores": [int(x) for x in _tc.split(",")]} if _tc else {}
    res = bass_utils.run_bass_kernel_spmd(nc, in_maps, core_ids=list(range(N_CORES)), **_kw)
    LAST_RESULTS = res

    out = np.empty((B, T, E), dtype=np.float32)
    for c in range(N_CORES):
        b, half = c // 2, c % 2
        out[b, half * TPC:(half + 1) * TPC, :] = res.results[c]["out"]
    return out


# revision 18
# speedup vs baseline: 1.1782x; 1.1782x over previous
"""Multi-head attention (no softmax) on 8 trn2 NeuronCores.

Reference: out = ((x @ Wqkv.T -> q,k,v per head) ; (q @ k.T * s) @ v ; concat ; @ Wproj.T)

Because there is no softmax the attention is linear:
    (q @ k.T) @ v == q @ (k.T @ v),  k.T @ v is only 64x64 per head,
so the T x T score matrices never need to exist. Per head:
    M_h = (s * k_h).T @ v_h        (64 x 64, reduced over ALL tokens of the batch)
    out += (q_h @ M_h) @ Wproj_h.T

Sharding: token-parallel. Core c owns batch b=c//2, token half c%2 (512 tokens).
M_h needs a reduction over the full batch -> two tiny 64KB AllGathers between
the two cores of each batch (peer-add done locally on DVE), overlapped with
the second kv half and the q matmuls.

The whole datapath runs in bf16 (fp32 PSUM accumulation): rel err ~5e-3 vs
the 2e-2 gate, and it halves HBM traffic (~10MB/core) so the kernel is
tensor-bound. The head-dim scale 1/8 is folded into W_k on the host (exact).

All inputs are host-packed into [128, N] DRAM tensors whose rows are fully
contiguous (4KB per partition line), so every DMA runs near full HBM rate.
One SBUF tile per DMA (whole-tile write deps would otherwise serialize
back-to-back transfers into the same tile):
  xP   (128, 4096):  xP[p, 512e+t]         = x[tok t, feat 128e+p]
  wkvP (128, 16384): wkvP[p, (P*8+e)512+c] = chunk_P.T[128e+p, c], chunks in
                     order (k half0, v half0, k half1, v half1), features
                     grouped h*64+j, k pre-scaled by 1/8
  wqP  (128, 8192):  wqP[p, 1024e+c]       = Wq_g.T[128e+p, c]
  wpP  (128, 8192):  wpP[p, 1024f+o]       = W_proj.T[128f+p, o]

Queues: Sync = kv weight stream then collective readbacks (idle by then);
GpSimd = x, wq, wp, gather bounces + triggers, output stores; Scalar(ACT) =
half the PSUM evictions + M staging copies; DVE = the other evictions + M
peer-adds. A chain of dummy matmuls at program start ramps the PE p-state
to 2.4GHz before real work arrives.

PSUM: one 8-bank rotating pool; allocation order keeps the 8 output
accumulators simultaneously live at the end.
"""

import numpy as np

B, T, E = 4, 1024, 1024
NH, HD = 16, 64
N_CORES = 8
TPC = T // 2  # tokens per core = 512

_built = None


def _build():
    """Build + compile the 8-core SPMD Bass program once."""
    global _built
    if _built is not None:
        return _built

    import concourse.mybir as mybir
    import concourse.tile as tile
    from concourse import bacc

    f32 = mybir.dt.float32
    bf16 = mybir.dt.bfloat16
    GROUPS = [[0, 1], [2, 3], [4, 5], [6, 7]]
    ALL8 = [[0, 1, 2, 3, 4, 5, 6, 7]]

    nc = bacc.Bacc("TRN2", target_bir_lowering=False, debug=False, num_devices=N_CORES)
    xP = nc.dram_tensor("xP", [128, 8 * TPC], bf16, kind="ExternalInput").ap()
    wkvP = nc.dram_tensor("wkvP", [128, 4 * 4096], bf16, kind="ExternalInput").ap()
    wqP = nc.dram_tensor("wqP", [128, 8 * 1024], bf16, kind="ExternalInput").ap()
    wpP = nc.dram_tensor("wpP", [128, 8 * 1024], bf16, kind="ExternalInput").ap()
    out = nc.dram_tensor("out", [TPC, E], f32, kind="ExternalOutput").ap()

    def evict(i, dst, src):
        # spread PSUM->SBUF eviction copies across DVE and ACT
        if i % 2 == 0:
            nc.vector.tensor_copy(dst, src)
        else:
            nc.scalar.copy(dst, src)

    with tile.TileContext(nc) as tc:
        with (
            tc.tile_pool(name="xp", bufs=1) as xp,
            tc.tile_pool(name="wkvp", bufs=1) as wkvp,
            tc.tile_pool(name="kvp", bufs=1) as kvp,
            tc.tile_pool(name="wqp", bufs=1) as wqp,
            tc.tile_pool(name="wpp", bufs=1) as wpp,
            tc.tile_pool(name="qp", bufs=1) as qp,
            tc.tile_pool(name="mres", bufs=1) as mres,
            tc.tile_pool(name="op", bufs=3) as op,
            tc.tile_pool(name="dram", bufs=1, space="DRAM") as dram,
            tc.tile_pool(name="ps", bufs=8, space="PSUM") as ps_pool,
        ):
            # ---- PE warmup: ramp the p-state while the preamble/DMAs run ----
            scratch = xp.tile([128, 640], bf16, tag="scratch")
            nc.vector.memset(scratch[:].bitcast(f32), 0.0)
            wps = ps_pool.tile([128, 512], f32, tag="big", name="warmup")
            for _ in range(9):
                nc.tensor.matmul(wps[:], scratch[:, 0:128], scratch[:, 128:640],
                                 start=True, stop=True)

            # ---- input DMAs: one SBUF tile per transfer ----
            # Sync: kv weight stream (fc order k0, v0, k1, v1; 2 tiles per fc).
            # GpSimd: x then wq (wp is issued later, after the first gather's
            # trigger, so the trigger isn't queued behind it). Scalar/DVE carry
            # NO transfers: a DMA trigger occupies its queue for the whole
            # transfer and would block the PSUM evictions behind it.
            # Split the kv stream across both rings: sync carries k0/v0 (the
            # first gather's inputs) then wq/wp; gpsimd carries x (FIRST -- the
            # ring runs transfers in issue order and every kv matmul needs x)
            # then k1/v1.
            xsb = []  # 2 tiles of [128, 2048]: e 0-3, e 4-7
            for h in range(2):
                t = xp.tile([128, 2048], bf16, tag=f"x{h}")
                nc.gpsimd.dma_start(t[:], xP[:, 2048 * h:2048 * (h + 1)])
                xsb.append(t)
            wkvsb = []  # 8 tiles of [128, 2048]: idx 2*pos + (e>=4)
            for h in range(8):
                t = wkvp.tile([128, 2048], bf16, tag=f"wkv{h}")
                eng = nc.sync if h < 4 else nc.gpsimd
                eng.dma_start(t[:], wkvP[:, 2048 * h:2048 * (h + 1)])
                wkvsb.append(t)
            wq = []  # 2 tiles of [128, 4096]: e 0-3, e 4-7
            for h in range(2):
                t = wqp.tile([128, 4096], bf16, tag=f"wq{h}")
                nc.sync.dma_start(t[:], wqP[:, 4096 * h:4096 * (h + 1)])
                wq.append(t)
            wp = []  # 2 tiles of [128, 4096]: f 0-3, f 4-7
            for h in range(2):
                t = wpp.tile([128, 4096], bf16, tag=f"wp{h}")
                nc.sync.dma_start(t[:], wpP[:, 4096 * h:4096 * (h + 1)])
                wp.append(t)

            def xs(e, lo, hi):  # x slice [128, hi-lo] for E-chunk e, token cols lo:hi
                return xsb[e // 4][:, 512 * (e % 4) + lo:512 * (e % 4) + hi]

            def wkvs(pos, e):  # wkv slice [128, 512] for fc position pos, E-chunk e
                return wkvsb[2 * pos + e // 4][:, 512 * (e % 4):512 * (e % 4 + 1)]

            kvsb = [kvp.tile([128, 2 * E], bf16, tag=f"kv{tt}", name=f"kv{tt}")
                    for tt in range(4)]
            Mbd = mres.tile([128, 1024], bf16, tag="Mbd")
            nc.vector.memset(Mbd[:].bitcast(f32), 0.0)

            bout = [None, None]

            # wkvP chunk position of each fc: fc0->0, fc2->1, fc1->2, fc3->3
            FCPOS = {0: 0, 2: 1, 1: 2, 3: 3}

            def kv_quarter(fc):
                pos = FCPOS[fc]
                for tt in range(4):
                    ps = ps_pool.tile([128, 512], f32, tag="big")
                    for e in range(8):
                        nc.tensor.matmul(
                            ps[:],
                            xs(e, 128 * tt, 128 * (tt + 1)),
                            wkvs(pos, e),
                            start=(e == 0), stop=(e == 7),
                        )
                    evict(tt, kvsb[tt][:, 512 * fc:512 * (fc + 1)], ps[:])

            def m_half(g):
                # M blocks 4g..4g+3 from k cols [512g:512g+512], v cols
                # [E+512g : E+512g+512]; keep only diagonal 64x64 sub-blocks.
                mp = ps_pool.tile([128, 512], f32, tag="big", name=f"mp{g}")
                for j in range(4):
                    blk = 4 * g + j
                    for tt in range(4):
                        nc.tensor.matmul(
                            mp[:, 128 * j:128 * (j + 1)],
                            kvsb[tt][:, 128 * blk:128 * (blk + 1)],
                            kvsb[tt][:, E + 128 * blk:E + 128 * (blk + 1)],
                            start=(tt == 0), stop=(tt == 3),
                        )
                Msb = mres.tile([128, 256], bf16, tag=f"Msb{g}", name=f"Msb{g}")
                for j in range(4):
                    nc.scalar.copy(Msb[0:64, 64 * j:64 * j + 64],
                                   mp[0:64, 128 * j:128 * j + 64])
                    nc.scalar.copy(Msb[64:128, 64 * j:64 * j + 64],
                                   mp[64:128, 128 * j + 64:128 * (j + 1)])
                # bounce to DRAM; bounce+trigger ride GpSimd, readbacks Sync
                bin_ = dram.tile([128, 256], bf16, name=f"bin{g}")
                bo = dram.tile([256, 256], bf16, name=f"bout{g}")
                nc.scalar.dma_start(bin_[:], Msb[:])
                nc.gpsimd.collective_compute(
                    "AllGather", mybir.AluOpType.bypass, replica_groups=GROUPS,
                    ins=[bin_.opt()], outs=[bo.opt()],
                )
                MrA = mres.tile([128, 256], bf16, tag=f"MrA{g}", name=f"MrA{g}")
                MrB = mres.tile([128, 256], bf16, tag=f"MrB{g}", name=f"MrB{g}")
                nc.sync.dma_start(MrA[:], bo[0:128, :])
                nc.sync.dma_start(MrB[:], bo[128:256, :])
                bout[g] = (MrA, MrB)

            def m_post(g):
                # add both ranks' partials straight into Mbd diagonal spots
                MrA, MrB = bout[g]
                for j in range(4):
                    blk = 4 * g + j
                    nc.vector.tensor_add(
                        Mbd[0:64, 128 * blk:128 * blk + 64],
                        MrA[0:64, 64 * j:64 * j + 64],
                        MrB[0:64, 64 * j:64 * j + 64])
                    nc.vector.tensor_add(
                        Mbd[64:128, 128 * blk + 64:128 * (blk + 1)],
                        MrA[64:128, 64 * j:64 * j + 64],
                        MrB[64:128, 64 * j:64 * j + 64])

            # ---- kv + M + gathers, pipelined in halves ----
            kv_quarter(0)      # k cols 0:512
            kv_quarter(2)      # v cols 0:512
            m_half(0)          # M blocks 0-3 + AllGather #1 (in flight)
            kv_quarter(1)      # k cols 512:1024
            kv_quarter(3)      # v cols 512:1024
            m_half(1)          # M blocks 4-7 + AllGather #2 (in flight)

            # ---- q (feature-major qT, (1024f, 512t)), overlaps the gathers ----
            qsb = [qp.tile([128, TPC], bf16, tag=f"q{f}", name=f"q{f}")
                   for f in range(8)]
            for fq in range(8):
                ps = ps_pool.tile([128, 512], f32, tag="big")
                for e in range(8):
                    nc.tensor.matmul(
                        ps[:],
                        wq[e // 4][:, 1024 * (e % 4) + 128 * fq:
                                   1024 * (e % 4) + 128 * (fq + 1)],
                        xs(e, 0, 512),
                        start=(e == 0), stop=(e == 7),
                    )
                evict(fq, qsb[fq][:], ps[:])

            # ---- att: attT_blk = Mbd_blk.T @ qT_blk (in-place into q tiles) ----
            m_post(0)
            for blk in range(4):
                ps = ps_pool.tile([128, 512], f32, tag="big")
                nc.tensor.matmul(ps[:], Mbd[:, 128 * blk:128 * (blk + 1)],
                                 qsb[blk][:], start=True, stop=True)
                evict(blk, qsb[blk][:], ps[:])
            attsb = qsb

            # ---- out = attT.T @ wpP  ((512t, 1024o)) ----
            # Two groups of 4 accumulators. Group A accumulates its att 0-3
            # contributions while gather #2 is still in flight; att 4-7 (the
            # only work gated on the gather) runs in the shadow of that, then
            # group A finishes, then group B does all 8 in order.
            def out_mm(ps, f, tt, oc, start, stop):
                nc.tensor.matmul(
                    ps[:],
                    attsb[f][:, 128 * tt:128 * (tt + 1)],
                    wp[f // 4][:, 1024 * (f % 4) + 512 * oc:
                               1024 * (f % 4) + 512 * (oc + 1)],
                    start=start, stop=stop,
                )

            groupA = [(0, 0), (0, 1), (1, 0), (1, 1)]
            groupB = [(2, 0), (2, 1), (3, 0), (3, 1)]
            psA = {}
            for tt, oc in groupA:
                ps = ps_pool.tile([128, 512], f32, tag="big", name=f"oA{tt}{oc}")
                psA[(tt, oc)] = ps
                for f in range(4):
                    out_mm(ps, f, tt, oc, start=(f == 0), stop=False)

            m_post(1)
            for blk in range(4, 8):
                ps = ps_pool.tile([128, 512], f32, tag="big")
                nc.tensor.matmul(ps[:], Mbd[:, 128 * blk:128 * (blk + 1)],
                                 qsb[blk][:], start=True, stop=True)
                evict(blk, qsb[blk][:], ps[:])

            i = 0
            for tt, oc in groupA:
                ps = psA[(tt, oc)]
                for f in range(4, 8):
                    out_mm(ps, f, tt, oc, start=False, stop=(f == 7))
                ot = op.tile([128, 512], f32, tag="osb")
                evict(i, ot[:], ps[:])
                eng = nc.gpsimd if i % 2 == 0 else nc.sync
                i += 1
                eng.dma_start(
                    out[128 * tt:128 * (tt + 1), 512 * oc:512 * (oc + 1)], ot[:])
            for tt, oc in groupB:
                ps = ps_pool.tile([128, 512], f32, tag="big", name=f"oB{tt}{oc}")
                for f in range(8):
                    out_mm(ps, f, tt, oc, start=(f == 0), stop=(f == 7))
                ot = op.tile([128, 512], f32, tag="osb")
                evict(i, ot[:], ps[:])
                eng = nc.gpsimd if i % 2 == 0 else nc.sync
                i += 1
                eng.dma_start(
                    out[128 * tt:128 * (tt + 1), 512 * oc:512 * (oc + 1)], ot[:])

    nc.compile()
    _built = nc
    return nc


LAST_RESULTS = None  # BassKernelResults of the most recent kernel() call


def kernel(x: np.ndarray, W_qkv: np.ndarray, W_proj: np.ndarray) -> np.ndarray:
    global LAST_RESULTS
    import ml_dtypes
    from concourse import bass_utils

    nc = _build()
    bf16 = ml_dtypes.bfloat16

    x = np.ascontiguousarray(x, dtype=np.float32)
    W_qkv = np.ascontiguousarray(W_qkv, dtype=np.float32)
    W_proj = np.ascontiguousarray(W_proj, dtype=np.float32)

    def pack(a, width):
        # (E, width) feature-major -> [128, 8*width] with row 128e+p -> col e*width
        return np.ascontiguousarray(
            a.reshape(8, 128, width).transpose(1, 0, 2).reshape(128, 8 * width)
            .astype(bf16))

    # head-grouping permutation: grouped feature h*64+j <- original row j*16+h
    perm = np.arange(E).reshape(HD, NH).T.ravel()
    Wq_g = W_qkv[perm]
    Wk_g = W_qkv[E + perm] * np.float32(HD ** -0.5)  # exact: 1/8
    Wv_g = W_qkv[2 * E + perm]
    K = Wk_g.T  # (E, kfeat)
    V = Wv_g.T
    wkvP_np = np.ascontiguousarray(np.concatenate(
        [pack(K[:, 0:512], 512), pack(V[:, 0:512], 512),
         pack(K[:, 512:1024], 512), pack(V[:, 512:1024], 512)], axis=1))
    wqP_np = pack(Wq_g.T, 1024)
    wpP_np = pack(W_proj.T, 1024)

    in_maps = []
    for c in range(N_CORES):
        b, half = c // 2, c % 2
        x_c = x[b, half * TPC:(half + 1) * TPC, :]  # (512 tok, E)
        xP_c = np.ascontiguousarray(
            x_c.reshape(TPC, 8, 128).transpose(2, 1, 0).reshape(128, 8 * TPC)
            .astype(bf16))
        in_maps.append({"xP": xP_c, "wkvP": wkvP_np, "wqP": wqP_np, "wpP": wpP_np})

    import os as _os
    _tc = _os.environ.get("KERNEL_TRACE_CORES")
    _kw = {"trace_cores": [int(x) for x in _tc.split(",")]} if _tc else {}
    res = bass_utils.run_bass_kernel_spmd(nc, in_maps, core_ids=list(range(N_CORES)), **_kw)
    LAST_RESULTS = res

    out = np.empty((B, T, E), dtype=np.float32)
    for c in range(N_CORES):
        b, half = c // 2, c % 2
        out[b, half * TPC:(half + 1) * TPC, :] = res.results[c]["out"]
    return out
